# revision 12
# baseline (speedup 1.0000x reference)
"""LoRA Linear (x @ W.T + b + scaling * (x @ A.T) @ B.T) on 8 TRN2 NeuronCores.

Strategy (fp8 DoubleRow):
  - Data-parallel: 8192 tokens -> 8 x 1024 rows, one shard per core.
  - Base matmul in fp8 e4m3 with MatmulPerfMode.DoubleRow (2 k-rows per
    partition per instruction, 0.5 cycles per output row).
  - Precision: fp8 quantization noise of x and W alone gives ~1.4% rel err,
    but ONLY if the LoRA adapter is NOT folded into W (folded-fp8 is ~3.9%:
    the rank-16 adapter dominates the output and would amplify both x's and
    W's quantization error). So the adapter path runs in bf16 on device:
        xa = x_bf16 @ A.T           (PE, 16-wide moving free dim -> ~free)
        xaT = transpose(xa)         (PE transpose via identity)
        adj = xaT.T @ [2*B.T; b]    (one bf16 matmul closing each PSUM
                                     accumulation group; row 16 of xaT is
                                     ones so the bias rides along)
  - Scales: x*32 and W*2048 keep fp8 values out of the subnormal range
    (max ~173 < 240). PSUM holds 65536*out; host divides by 2^16 (exact).
  - Output bf16 (halves out DMA), fp32 conversion on host.
  - DMA: few large transfers (per-DMA HWDGE prep is ~625ns exclusive):
    one DMA per W o-block, one per x m-tile per dtype, one per output
    column (column-staging SBUF tile). Single SP queue; slot-gated W
    DMAs (o-blocks 3..7, triple-buffered tag) are enqueued only after
    everything needed to free their slot (FIFO deadlock otherwise).
  - PE order: o-blocks 0,1 interleaved as a 2-column band while x still
    streams in, then o-blocks 2..7 column-major.
"""

import numpy as np
import ml_dtypes

import concourse.bass as bass
from concourse import bacc
import concourse.mybir as mybir
import concourse.tile as tile
from concourse.bass_utils import run_bass_kernel_spmd

N_CORES = 8
IN_F = 4096
OUT_F = 4096
RANK = 16
ALPHA = 32.0
B_SZ = 4
S_SZ = 2048
TOK = B_SZ * S_SZ            # 8192
M_PER_CORE = TOK // N_CORES  # 1024

P = 128                      # partitions
KT = IN_F // P               # 32 k-tiles
KP = KT // 2                 # 16 k-pair tiles (DoubleRow)
O_BLK = 512                  # o-block width (psum bank = 512 fp32)
N_OBLK = OUT_F // O_BLK      # 8
MT = M_PER_CORE // P         # 8 m-tiles
RP = RANK + 1                # rank rows + ones row (bias)

SX = 32.0                    # x fp8 scale
SW = 2048.0                  # W fp8 scale
DESCALE = 1.0 / (SX * SW)    # applied on host (power of two, exact)

F8 = mybir.dt.float8e4
BF = mybir.dt.bfloat16
NP_F8 = ml_dtypes.float8_e4m3
NP_BF = ml_dtypes.bfloat16

LAST_RESULTS = None          # test.py reads exec_time_ns from here
_NC_CACHE = None


def _default_schedule():
    # Tuple tokens: ("at",)/("bb",)/("id",) constants, ("f8", i[, half]) /
    # ("bf", i[, half]) x m-tiles, ("w", j[, half]) W o-blocks.
    dma_seq = [("at",), ("f8", 0), ("bf", 0), ("id",), ("bb",), ("w", 0),
               ("f8", 1), ("bf", 1), ("f8", 2), ("bf", 2), ("w", 1),
               ("f8", 3), ("bf", 3), ("f8", 4), ("bf", 4), ("w", 2),
               ("f8", 5), ("bf", 5), ("f8", 6), ("bf", 6), ("f8", 7),
               ("bf", 7)]
    lags = (0, 1, 4)
    ops = []
    for i in range(MT):
        ops.append(("xa", i))
        for j, lag in enumerate(lags):
            if 0 <= i - lag < MT:
                ops.append(("g", i - lag, j))
    for j, lag in enumerate(lags):
        for i in range(MT - lag, MT):
            ops.append(("g", i, j))
    for j in range(len(lags), N_OBLK):
        for i in range(MT):
            ops.append(("g", i, j))
    pe_seq = [("d", 30), ops[0]] + ops[1:]
    return dma_seq, pe_seq


def _build_nc(schedule=None):
    dma_seq, pe_seq = schedule if schedule is not None else _default_schedule()

    nc = bacc.Bacc(None, target_bir_lowering=False)

    xf8_d = nc.dram_tensor("xf8", [MT, P, KT, P], F8, kind="ExternalInput")
    xbf_d = nc.dram_tensor("xbf", [MT, P, KT, P], BF, kind="ExternalInput")
    wt_d = nc.dram_tensor("wt", [N_OBLK, P, KP, 2, O_BLK], F8,
                          kind="ExternalInput")
    at_d = nc.dram_tensor("at", [P, KT, RANK], BF, kind="ExternalInput")
    bb_d = nc.dram_tensor("bb", [RP, OUT_F], BF, kind="ExternalInput")
    id_d = nc.dram_tensor("ident", [P, P], BF, kind="ExternalInput")
    # [i][p][j][c] row-major == [1024, 4096] row-major
    out_d = nc.dram_tensor("out", [MT, P, N_OBLK, O_BLK], BF,
                           kind="ExternalOutput")

    with tile.TileContext(nc) as tc:
        with (
            tc.tile_pool(name="xp", bufs=1) as xp,
            tc.tile_pool(name="wp", bufs=3) as wp,
            tc.tile_pool(name="sm", bufs=1) as sm,
            tc.tile_pool(name="outs", bufs=8) as outs,
            tc.tile_pool(name="pmain", bufs=4, space="PSUM") as pmain,
            tc.tile_pool(name="pxa", bufs=2, space="PSUM") as pxa,
            tc.tile_pool(name="ptr", bufs=1, space="PSUM") as ptr,
        ):
            # ---- small constants ----
            at_sb = sm.tile([P, KT, RANK], BF, tag="at")
            bb_sb = sm.tile([RP, OUT_F], BF, tag="bb")
            id_sb = sm.tile([P, P], BF, tag="ident")
            warm = sm.tile([P, P], BF, tag="warm")
            nc.vector.memset(warm[:], 0.0)
            xaT = sm.tile([RP, M_PER_CORE], BF, tag="xaT")
            nc.vector.memset(xaT[RANK:RP, :], 1.0)
            wps = pxa.tile([P, O_BLK], mybir.dt.float32, tag="wps", bufs=1)
            smalls = {"at": (at_sb, at_d), "bb": (bb_sb, bb_d),
                      "id": (id_sb, id_d)}

            xf8s, xbfs = [], []
            for i in range(MT):
                xf8s.append(xp.tile([P, KT, P], F8, tag=f"xf8_{i}",
                                    name=f"xf8_{i}"))
                xbfs.append(xp.tile([P, KT, P], BF, tag=f"xbf_{i}",
                                    name=f"xbf_{i}"))

            w_tiles = {}
            col_tiles = {}
            emitted_w = set()

            def dummies(n):
                for _ in range(n):
                    nc.tensor.matmul(wps[:, 0:P], warm[:], warm[:],
                                     start=True, stop=True)

            def dma_w(j, half=None):
                if j >= N_OBLK:
                    return
                if j not in w_tiles:
                    w_tiles[j] = wp.tile([P, KP, 2, O_BLK], F8, tag="w",
                                         name=f"w{j}")
                t = w_tiles[j]
                if half is None:
                    if j in emitted_w:
                        return
                    emitted_w.add(j)
                    nc.sync.dma_start(t[:], wt_d[j])
                else:
                    h = KP // 2
                    sl = slice(half * h, (half + 1) * h)
                    emitted_w.add(j)
                    nc.sync.dma_start(t[:, sl, :, :], wt_d[j, :, sl, :, :])

            def xa(i):
                # xa_psum[tok, r] = sum_k x_bf16[tok, k] * A[r, k]
                ps = pxa.tile([P, RANK], mybir.dt.float32, tag="pxa",
                              name=f"pxa{i}")
                for k in range(KT):
                    nc.tensor.matmul(
                        ps[:], xbfs[i][:, k, :], at_sb[:, k, :],
                        start=(k == 0), stop=(k == KT - 1),
                    )
                xa_sb = sm.tile([P, RANK], BF, tag="xa_sb", bufs=2,
                                name=f"xa_sb{i}")
                nc.scalar.copy(xa_sb[:], ps[:])
                tr = ptr.tile([RANK, P], BF, tag="ptr", name=f"ptr{i}")
                nc.tensor.transpose(tr[:], xa_sb[:], id_sb[:])
                nc.scalar.copy(xaT[0:RANK, i * P:(i + 1) * P], tr[:])

            gidx = 0
            col_remaining = {j: MT for j in range(N_OBLK)}

            def group(i, j):
                nonlocal gidx
                ps = pmain.tile([P, O_BLK], mybir.dt.float32, tag="pm",
                                name=f"pm{i}_{j}")
                for kp in range(KP):
                    nc.tensor.matmul(
                        ps[:], xf8s[i][:, 2 * kp:2 * kp + 2, :],
                        w_tiles[j][:, kp, :, :],
                        start=(kp == 0), stop=False,
                        perf_mode=mybir.MatmulPerfMode.DoubleRow,
                    )
                nc.tensor.matmul(
                    ps[:], xaT[:, i * P:(i + 1) * P],
                    bb_sb[:, j * O_BLK:(j + 1) * O_BLK],
                    start=False, stop=True,
                )
                o_sb = outs.tile([P, O_BLK], BF, tag="osb", name=f"osb{i}_{j}")
                if gidx % 2 == 0:
                    nc.scalar.copy(o_sb[:], ps[:])
                else:
                    nc.vector.tensor_copy(o_sb[:], ps[:])
                gidx += 1
                nc.scalar.dma_start(out_d[i, :, j, :], o_sb[:])
                col_remaining[j] -= 1
                if col_remaining[j] == 0:
                    dma_w(j + 3)

            for tok in dma_seq:
                kind = tok[0]
                if kind in smalls:
                    t, d = smalls[kind]
                    nc.sync.dma_start(t[:], d[:])
                elif kind in ("bf", "f8"):
                    i, half = tok[1], tok[2] if len(tok) > 2 else None
                    tl = xbfs[i] if kind == "bf" else xf8s[i]
                    dr = xbf_d if kind == "bf" else xf8_d
                    if half is None:
                        nc.sync.dma_start(tl[:], dr[i])
                    else:
                        h = KT // 2
                        sl = slice(half * h, (half + 1) * h)
                        nc.sync.dma_start(tl[:, sl, :], dr[i, :, sl, :])
                elif kind == "w":
                    dma_w(tok[1], tok[2] if len(tok) > 2 else None)
            for op in pe_seq:
                if op[0] == "xa":
                    xa(op[1])
                elif op[0] == "d":
                    dummies(op[1])
                else:
                    group(op[1], op[2])
    nc.compile()
    return nc


def _prep_inputs(x, W, b, lora_A, lora_B):
    scaling = ALPHA / RANK
    Wq = (W.astype(np.float32) * SW).astype(NP_F8)            # [out, in]
    # wt[ob, p, kp, t, c] = Wq[ob*512 + c, (2*kp + t)*128 + p]
    wt_in = np.ascontiguousarray(
        Wq.T.reshape(KP, 2, P, N_OBLK, O_BLK).transpose(3, 2, 0, 1, 4)
    )

    at_in = np.ascontiguousarray(
        lora_A.astype(np.float32).T.reshape(KT, P, RANK).transpose(1, 0, 2)
    ).astype(NP_BF)                                           # [p, k, r]

    bb_in = np.zeros((RP, OUT_F), dtype=NP_BF)
    bb_in[0:RANK] = (lora_B.astype(np.float32).T * (scaling * SX * SW)).astype(NP_BF)
    bb_in[RANK] = (b.astype(np.float32) * (SX * SW)).astype(NP_BF)

    id_in = np.eye(P, dtype=NP_BF)

    x_flat = np.ascontiguousarray(x.reshape(TOK, IN_F).astype(np.float32))
    in_maps = []
    for c in range(N_CORES):
        xc = x_flat[c * M_PER_CORE:(c + 1) * M_PER_CORE]      # [1024, 4096]
        # x[m, p, k, c] = xc[m*128 + c, k*128 + p]
        xt = xc.T.reshape(KT, P, MT, P).transpose(2, 1, 0, 3)
        xf8_in = np.ascontiguousarray(xt * SX).astype(NP_F8)
        xbf_in = np.ascontiguousarray(xt).astype(NP_BF)
        in_maps.append({
            "xf8": xf8_in,
            "xbf": xbf_in,
            "wt": wt_in,
            "at": at_in,
            "bb": bb_in,
            "ident": id_in,
        })
    return in_maps


def kernel(x, W, b, lora_A, lora_B, _trace=False):
    global LAST_RESULTS, _NC_CACHE

    in_maps = _prep_inputs(x, W, b, lora_A, lora_B)

    if _NC_CACHE is None:
        _NC_CACHE = _build_nc()
    nc = _NC_CACHE

    res = run_bass_kernel_spmd(nc, in_maps, core_ids=list(range(N_CORES)),
                               trace=_trace)
    LAST_RESULTS = res

    out = np.concatenate(
        [r["out"].reshape(M_PER_CORE, OUT_F).astype(np.float32)
         for r in res.results], axis=0)
    out *= np.float32(DESCALE)
    return out.reshape(B_SZ, S_SZ, OUT_F).astype(np.float32)
